# revision 3
# baseline (speedup 1.0000x reference)
"""Causal self-attention (GPT-2 small shape) on 8 Trainium2 NeuronCores.

Data-parallel over batch: B=16 -> 2 batches per core, no collectives.

Per-core plan (T=1024, C=768, H=12, d=64), all heavy matmuls in float32r
(full-rate fp32 with TF32-ish mantissa rounding on the PE):

  x^T[C,T]   : PE transpose of x tiles (fp32), cast to f32r on copy-out
  qk^T       : W_attn[:, :1536].T @ x -> q^T,k^T in [feat, tok] layout;
               bias (+1/8 scale for q) fused into the PSUM->SBUF copy
  v_aug      : x @ [W_v | 0] + [b_v | 1]  -> [tok, 6*(d+1)] per half;
               ones column provides softmax denominators downstream
  S^T        : k_j^T.T @ q^T per (head, k-tile j), causal chunks only
  P^T        : exp on ScalarE (no max subtraction; scores are small),
               upper-tri mask multiply on the diagonal 128x128 block
  att@v      : y^T[65, qchunk] = [v_j | 1].T @ P^T accumulated over j;
               row 64 = softmax denominator
  normalize  : reciprocal(denom) -> broadcast over 64 partitions via a
               K=1 matmul -> y^T scaled and written into paired [128,T]
               tiles (partition-shifted writes for odd heads)
  proj       : out[tok, C] = y^T.T @ W_proj + b_proj (bias via K=1 matmul)
"""

import numpy as np

import concourse.bass as bass
import concourse.mybir as mybir
import concourse.tile as tile
from concourse import bacc
from concourse.bass_utils import run_bass_kernel_spmd

f32 = mybir.dt.float32
f32r = mybir.dt.float32r
AF = mybir.ActivationFunctionType
OP = mybir.AluOpType

N_CORES = 8
B, T, C = 16, 1024, 768
H, D = 12, 64
BL = B // N_CORES          # batches per core
NT = T // 128              # 8 token tiles per batch
KC = C // 128              # 6 contraction chunks
QCH = T // 512             # 2 q-chunks of 512


def build_nc():
    nc = bacc.Bacc("TRN2", target_bir_lowering=False, debug=False,
                   num_devices=N_CORES)

    x_d = nc.dram_tensor("x", [BL, T, C], f32, kind="ExternalInput").ap()
    wat_d = nc.dram_tensor("W_attn", [C, 3 * C], f32, kind="ExternalInput").ap()
    bat_d = nc.dram_tensor("b_attn", [3 * C], f32, kind="ExternalInput").ap()
    wpr_d = nc.dram_tensor("W_proj", [C, C], f32, kind="ExternalInput").ap()
    bpr_d = nc.dram_tensor("b_proj", [C], f32, kind="ExternalInput").ap()
    out_d = nc.dram_tensor("out", [BL, T, C], f32, kind="ExternalOutput").ap()

    ident_t = nc.inline_tensor(np.eye(128, dtype=np.float32), name="ident")
    # S^T tile layout is [tk, tq]; valid entries tk <= tq -> upper incl diag
    tri_t = nc.inline_tensor(np.triu(np.ones((128, 128), np.float32)),
                             name="triu")
    onesr_t = nc.inline_tensor(np.ones((1, 128), np.float32), name="onesr")
    onesc_t = nc.inline_tensor(np.ones((128, 6, 1), np.float32), name="onesc")
    zeroc_t = nc.inline_tensor(np.zeros((128, 6, 1), np.float32), name="zeroc")
    onesb_t = nc.inline_tensor(np.ones((1, 6, 1), np.float32), name="onesb")

    with tile.TileContext(nc) as tc:
        build_body(nc, tc, x_d, wat_d, bat_d, wpr_d, bpr_d, out_d,
                   ident_t, tri_t, onesr_t, zeroc_t, onesb_t)
    nc.compile()
    return nc


def build_body(nc, tc, x_d, wat_d, bat_d, wpr_d, bpr_d, out_d,
               ident_t, tri_t, onesr_t, zeroc_t, onesb_t):
    import contextlib
    ctx = contextlib.ExitStack()
    with ctx:
        consts = ctx.enter_context(tc.tile_pool(name="consts", bufs=1))
        wqk_p = ctx.enter_context(tc.tile_pool(name="wqk", bufs=1))
        wv_p = ctx.enter_context(tc.tile_pool(name="wv", bufs=1))
        wpr_p = ctx.enter_context(tc.tile_pool(name="wpr", bufs=1))
        xn_p = ctx.enter_context(tc.tile_pool(name="xn", bufs=2))
        xt_p = ctx.enter_context(tc.tile_pool(name="xt", bufs=1))
        qk_p = ctx.enter_context(tc.tile_pool(name="qk", bufs=1))
        va_p = ctx.enter_context(tc.tile_pool(name="va", bufs=2))
        pt_p = ctx.enter_context(tc.tile_pool(name="pt", bufs=1))
        yt_p = ctx.enter_context(tc.tile_pool(name="yt", bufs=1))
        sm_p = ctx.enter_context(tc.tile_pool(name="sm", bufs=2))
        ob_p = ctx.enter_context(tc.tile_pool(name="ob", bufs=2))
        ps = ctx.enter_context(tc.tile_pool(name="ps", bufs=4, space="PSUM"))
        psy = ctx.enter_context(tc.tile_pool(name="psy", bufs=2, space="PSUM"))
        psb = ctx.enter_context(tc.tile_pool(name="psb", bufs=1, space="PSUM"))

        # ---- constants ----
        ident = consts.tile([128, 128], f32)
        tri = consts.tile([128, 128], f32r)
        ones_row = consts.tile([1, 128], f32r)   # lhsT for K=1 bias matmuls
        b_qk = consts.tile([128, 12], f32)       # per-partition qk biases
        b_pr = consts.tile([1, C], f32r)
        nc.sync.dma_start(out=ident, in_=ident_t.ap())
        nc.sync.dma_start(out=tri, in_=tri_t.ap().bitcast(f32r))
        nc.sync.dma_start(out=ones_row, in_=onesr_t.ap().bitcast(f32r))
        nc.sync.dma_start(out=b_qk,
                          in_=bat_d[0:1536].rearrange("(f p) -> p f", p=128))
        # pre-scale q biases by 1/8 (activation applies scale to input only)
        nc.vector.tensor_scalar_mul(b_qk[:, 0:6], b_qk[:, 0:6], 0.125)
        nc.sync.dma_start(out=b_pr,
                          in_=bpr_d.bitcast(f32r).rearrange("(o c) -> o c", o=1))

        # ---- resident weights ----
        w_qk = []
        for c in range(KC):
            wt = wqk_p.tile([128, 1536], f32r, name=f"wqk{c}")
            nc.sync.dma_start(
                out=wt, in_=wat_d.bitcast(f32r)[c * 128:(c + 1) * 128, 0:1536])
            w_qk.append(wt)
        w_pr = []
        for c in range(KC):
            wt = wpr_p.tile([128, C], f32r, name=f"wpr{c}")
            nc.sync.dma_start(
                out=wt, in_=wpr_d.bitcast(f32r)[c * 128:(c + 1) * 128, :])
            w_pr.append(wt)

        for b in range(BL):
            with nc.named_scope(f"xpose_b{b}"):
                x_t = xpose(nc, xn_p, xt_p, ps, x_d, ident, b)
            y_t = [yt_p.tile([128, T], f32r, tag=f"yt{f}", name=f"yt{b}_{f}")
                   for f in range(KC)]
            for half in range(2):
                with nc.named_scope(f"qkv_b{b}h{half}"):
                    qk_t, va_t = qkv_half(nc, tc, qk_p, va_p, wv_p, consts, ps,
                                          x_t, w_qk, b_qk, wat_d, bat_d,
                                          zeroc_t, onesb_t, ones_row, b, half)
                with nc.named_scope(f"attn_b{b}h{half}"):
                    for hh in range(6):
                        attn_head(nc, tc, pt_p, sm_p, ps, psy, psb,
                                  qk_t, va_t, y_t, tri, ones_row,
                                  b, half, hh)
            with nc.named_scope(f"proj_b{b}"):
                proj(nc, ob_p, ps, y_t, w_pr, b_pr, ones_row, out_d, b)


def xpose(nc, xn_p, xt_p, ps, x_d, ident, b):
    """x[b] natural -> x^T tiles [128, T] f32r, one per C-chunk."""
    x_t = [xt_p.tile([128, T], f32r, tag=f"xt{c}", name=f"xt{b}_{c}")
           for c in range(KC)]
    for t in range(NT):
        xn = xn_p.tile([128, C], f32, name="xn")
        nc.sync.dma_start(out=xn, in_=x_d[b, t * 128:(t + 1) * 128, :])
        for c in range(KC):
            tp = ps.tile([128, 128], f32, tag="mm", name="tp")
            nc.tensor.transpose(tp, xn[:, c * 128:(c + 1) * 128], ident)
            nc.vector.tensor_copy(out=x_t[c][:, t * 128:(t + 1) * 128],
                                  in_=tp)
    return x_t


def qkv_half(nc, tc, qk_p, va_p, wv_p, consts, ps, x_t, w_qk, b_qk,
             wat_d, bat_d, zeroc_t, onesb_t, ones_row, b, half):
    """q^T,k^T pair tiles + v_aug tiles for heads [6*half, 6*half+6)."""
    # --- v_aug weights for this half: [128, 6, 65] per C-chunk ---
    w_va = []
    for c in range(KC):
        wv = wv_p.tile([128, 6, 65], f32r, tag=f"wva{c}", name=f"wva{c}")
        nc.sync.dma_start(
            out=wv[:, :, 0:64],
            in_=wat_d.bitcast(f32r)[c * 128:(c + 1) * 128,
                                    1536 + half * 384:1536 + half * 384 + 384
                                    ].rearrange("p (h d) -> p h d", d=64))
        nc.sync.dma_start(out=wv[:, :, 64:65],
                          in_=zeroc_t.ap().bitcast(f32r))
        w_va.append(wv)
    b_va = consts.tile([1, 6, 65], f32r, tag="bva", bufs=2, name="bva")
    nc.sync.dma_start(
        out=b_va[:, :, 0:64],
        in_=bat_d.bitcast(f32r)[1536 + half * 384:1536 + half * 384 + 384
                                ].rearrange("(o h d) -> o h d", o=1, d=64))
    nc.sync.dma_start(out=b_va[:, :, 64:65], in_=onesb_t.ap().bitcast(f32r))

    # --- q^T / k^T tiles: 3 head-pairs, q fchunks half*3.., k 6+half*3.. ---
    qk_t = []
    for i, fc in enumerate([half * 3, half * 3 + 1, half * 3 + 2,
                            6 + half * 3, 6 + half * 3 + 1, 6 + half * 3 + 2]):
        qt = qk_p.tile([128, T], f32r, tag=f"qk{i}", name=f"qk{b}_{half}_{fc}")
        is_q = fc < 6
        for n in range(QCH):
            mp = ps.tile([128, 512], f32, tag="mm", name="mp")
            for c in range(KC):
                nc.tensor.matmul(
                    mp, w_qk[c][:, fc * 128:(fc + 1) * 128],
                    x_t[c][:, n * 512:(n + 1) * 512],
                    start=(c == 0), stop=(c == KC - 1))
            # bias add (+ 1/8 scale for q) fused into copy-out on ScalarE
            nc.scalar.activation(
                out=qt[:, n * 512:(n + 1) * 512], in_=mp,
                func=AF.Identity, bias=b_qk[:, fc:fc + 1],
                scale=0.125 if is_q else 1.0)
        qk_t.append(qt)

    # --- v_aug tiles: [128 tok, 6, 65] per token tile ---
    va_t = []
    for t in range(NT):
        va = va_p.tile([128, 6, 65], f32r, tag=f"va{t}", name=f"va{t}")
        vp = ps.tile([128, 390], f32, tag="mm", name="vp")
        for c in range(KC):
            nc.tensor.matmul(
                vp, x_t[c][:, t * 128:(t + 1) * 128],
                w_va[c].rearrange("p h d -> p (h d)"),
                start=(c == 0), stop=False)
        nc.tensor.matmul(vp, ones_row, b_va.rearrange("o h d -> o (h d)"),
                         start=False, stop=True)
        nc.vector.tensor_copy(out=va.rearrange("p h d -> p (h d)"), in_=vp)
        va_t.append(va)
    return qk_t, va_t


def causal_chunks(j):
    """Global q-chunk-aligned S^T chunks for k-tile j: (qc, q0, width)."""
    out = []
    for qc in range(QCH):
        q0 = max(qc * 512, j * 128)
        q1 = (qc + 1) * 512
        if q1 > q0:
            out.append((qc, q0, q1 - q0))
    return out


def attn_head(nc, tc, pt_p, sm_p, ps, psy, psb, qk_t, va_t, y_t, tri,
              ones_row, b, half, hh):
    h = half * 6 + hh                 # global head
    pair = hh // 2                    # index into qk_t (0..2)
    p01 = hh % 2                      # position within the 128-part pair tile
    qt = qk_t[pair]
    kt = qk_t[3 + pair]
    lo, hi = 64 * p01, 64 * p01 + 64

    # S^T -> exp -> P^T chunk tiles
    pt = {}
    for j in range(NT):
        for (qc, q0, w) in causal_chunks(j):
            sp = ps.tile([128, w], f32, tag="mm", name="sp")
            nc.tensor.matmul(sp, kt[lo:hi, j * 128:(j + 1) * 128],
                             qt[lo:hi, q0:q0 + w], start=True, stop=True)
            p = pt_p.tile([128, w], f32r, tag=f"pt{j}_{qc}", name=f"pt{j}_{qc}")
            nc.scalar.activation(out=p, in_=sp, func=AF.Exp)
            if q0 == j * 128:  # diagonal block: causal mask multiply
                nc.vector.tensor_tensor(out=p[:, 0:128], in0=p[:, 0:128],
                                        in1=tri, op=OP.mult)
            pt[(j, qc)] = p

    # att@v with ones-augmented v, then normalize
    for qc in range(QCH):
        js = [j for j in range(NT) if j * 128 < (qc + 1) * 512]
        yp = psy.tile([65, 512], f32, tag="y", name="yp")
        for i, j in enumerate(js):
            q0 = max(qc * 512, j * 128)
            off = q0 - qc * 512
            nc.tensor.matmul(yp[:, off:], va_t[j][:, hh, :], pt[(j, qc)],
                             start=(i == 0), stop=(i == len(js) - 1))
        recip = sm_p.tile([1, 512], f32r, tag="recip", name="recip")
        with nc.allow_low_precision(reason="f32r == f32 bits"):
            nc.vector.reciprocal(out=recip, in_=yp[64:65, :])
        bc = psb.tile([64, 512], f32, tag="bc", name="bc")
        nc.tensor.matmul(bc, ones_row[:, 0:64], recip, start=True, stop=True)
        bcs = sm_p.tile([64, 512], f32, tag="bcs", name="bcs")
        nc.scalar.copy(out=bcs, in_=bc)
        # normalized y^T written into the paired tile (partition shift for odd)
        nc.vector.tensor_tensor(
            out=y_t[half * 3 + pair][lo:hi, qc * 512:(qc + 1) * 512],
            in0=yp[0:64, :], in1=bcs, op=OP.mult)


def proj(nc, ob_p, ps, y_t, w_pr, b_pr, ones_row, out_d, b):
    for t in range(NT):
        ob = ob_p.tile([128, C], f32, name="ob")
        for n in range(2):
            pp = ps.tile([128, 384], f32, tag="mm", name="pp")
            for c in range(KC):
                nc.tensor.matmul(
                    pp, y_t[c][:, t * 128:(t + 1) * 128],
                    w_pr[c][:, n * 384:(n + 1) * 384],
                    start=(c == 0), stop=False)
            nc.tensor.matmul(pp, ones_row, b_pr[:, n * 384:(n + 1) * 384],
                             start=False, stop=True)
            nc.vector.tensor_copy(out=ob[:, n * 384:(n + 1) * 384], in_=pp)
        nc.sync.dma_start(out=out_d[b, t * 128:(t + 1) * 128, :], in_=ob)


_NC_CACHE = {}


def get_nc():
    if "nc" not in _NC_CACHE:
        _NC_CACHE["nc"] = build_nc()
    return _NC_CACHE["nc"]


def kernel(x, W_attn, b_attn, W_proj, b_proj):
    x = np.ascontiguousarray(np.asarray(x, dtype=np.float32))
    W_attn = np.ascontiguousarray(np.asarray(W_attn, dtype=np.float32))
    b_attn = np.ascontiguousarray(np.asarray(b_attn, dtype=np.float32))
    W_proj = np.ascontiguousarray(np.asarray(W_proj, dtype=np.float32))
    b_proj = np.ascontiguousarray(np.asarray(b_proj, dtype=np.float32))

    nc = get_nc()
    in_maps = []
    for i in range(N_CORES):
        in_maps.append({
            "x": x[i * BL:(i + 1) * BL],
            "W_attn": W_attn, "b_attn": b_attn,
            "W_proj": W_proj, "b_proj": b_proj,
        })
    res = run_bass_kernel_spmd(nc, in_maps, core_ids=list(range(N_CORES)))
    return np.concatenate([r["out"] for r in res.results], axis=0)
